# revision 24
# baseline (speedup 1.0000x reference)
"""GQA attention (B=1, E=4096, H=32, KVH=8, S=2048, HD=128) on 8 TRN2 cores.

Sharding: tensor-parallel over heads. Core c owns q heads {c, c+8, c+16, c+24}
(all attend to kv head c under the reference's channel-block-repeat GQA
tiling): 4 q-head projections + 1 kv-head k/v projection + RoPE + causal
attention are core-local. Attention outputs are AllGathered per head
(2 MiB/rank bf16); each core then computes a 512-row slice of o_proj.

Schedule (v3): keep the PE stream dense and post collectives early so the
per-rank SPMD launch skew (observed 50-100us) is absorbed by queued
independent work instead of PE idle:
  A1: k/v/q0/q1 projections (4 PSUM accumulators) for all 4 seq tiles.
  attn(h0) then attn(h1), each interleaved with A2 (q2/q3 projection) quanta
    as PE filler between score/AV pairs -> AG(h0)/AG(h1) post ~80us earlier
    than an h-major schedule would.
  attn(h2) drains A2 leftovers; attn(h3) pops o_proj(h0) quanta (AG0 landed
    unless skew is extreme; then it stalls no worse than the tail would).
  o_proj drain h0..h3 with per-(t,o) out DMAs.
Exp runs as two [128,512] halves per pair (halves the ACT latency the AV
waits on). Softmax denominator: DVE pair-folds + one ones-matmul per 4
key-blocks. Scores+exp always run full-width so PSUM/w_t hold finite values
everywhere (0*NaN from masking stale memory poisoned the denominator once);
only the AV matmul shrinks masked columns.

Numerics: matmuls bf16 (f32 PSUM); softmax without max-subtraction (scores are
O(5)); exp on ScalarE from PSUM with 1/sqrt(128) folded into activation scale.
"""

import numpy as np
import ml_dtypes

B, E, H, KVH, S = 1, 4096, 32, 8, 2048
HD = E // H            # 128
NCORES = 8
QH = H // NCORES       # 4 q heads per core
EB = E // 128          # 32 e-blocks
ST = S // 512          # 4 s-tiles of 512
JB = S // 128          # 16 j-blocks of 128
OCB = (E // NCORES) // 128  # 4 output-channel blocks per core (512 rows)

_BF16 = ml_dtypes.bfloat16

_COMPILED = None       # (nc, names) cache
LAST_EXEC_NS = None    # set when _profile=True
LAST_INSTS = None


def _build_graph():
    import concourse.bass as bass
    import concourse.bacc as bacc
    import concourse.mybir as mybir
    from concourse import tile
    from concourse.masks import make_identity

    f32 = mybir.dt.float32
    bf16 = mybir.dt.bfloat16
    SCALE = 1.0 / float(np.sqrt(HD))

    nc = bacc.Bacc("TRN2", target_bir_lowering=False, num_devices=NCORES)

    xp = nc.declare_dram_parameter("xp", [ST, 128, EB, 512], bf16, isOutput=False)
    wqt = nc.declare_dram_parameter("wqt", [128, EB, QH, 128], bf16, isOutput=False)
    wkt = nc.declare_dram_parameter("wkt", [128, EB, 128], bf16, isOutput=False)
    wvt = nc.declare_dram_parameter("wvt", [128, EB, 128], bf16, isOutput=False)
    wot = nc.declare_dram_parameter("wot", [128, EB, OCB, 128], bf16, isOutput=False)
    cosd = nc.declare_dram_parameter("cosd", [128, S], bf16, isOutput=False)
    sind = nc.declare_dram_parameter("sind", [128, S], bf16, isOutput=False)
    bvp = nc.declare_dram_parameter("bvp", [128, 1], f32, isOutput=False)
    bop = nc.declare_dram_parameter("bop", [128, OCB], f32, isOutput=False)
    out = nc.declare_dram_parameter("out", [OCB, ST, 128, 512], f32, isOutput=True)

    with tile.TileContext(nc) as tc:
        with (
            tc.tile_pool(name="const", bufs=1) as constp,
            tc.tile_pool(name="keep", bufs=1) as keep,
            tc.tile_pool(name="wtp", bufs=1) as wtp,
            tc.tile_pool(name="ftmp", bufs=2) as ftmp,
            tc.tile_pool(name="cw", bufs=1) as cw,
            tc.tile_pool(name="dramp", bufs=1, space="DRAM") as dramp,
        ):
            ident = constp.tile([128, 128], bf16)
            make_identity(nc, ident)
            ones = constp.tile([128, 128], bf16)
            nc.gpsimd.memset(ones[:], 1.0)
            # diag masks: mask_d[j, s] = 1 if s >= 128*d + j else 0
            dmask = constp.tile([128, 4, 512], bf16)
            for d in range(4):
                nc.gpsimd.memset(dmask[:, d, :], 1.0)
                nc.gpsimd.affine_select(
                    out=dmask[:, d, :], in_=dmask[:, d, :],
                    compare_op=mybir.AluOpType.is_ge,
                    fill=0.0, base=-128 * d,
                    pattern=[[1, 512]], channel_multiplier=-1,
                )
            cos_s = constp.tile([128, S], bf16)
            sin_s = constp.tile([128, S], bf16)
            bv_s = constp.tile([128, 1], f32)
            bo_s = constp.tile([128, OCB], f32)
            # zero the 4 rotating w_t buffers once: masked/never-exp'd columns
            # must read as 0.0 (stale SBUF can hold NaN bit patterns and the
            # dmask multiply would turn them into 0*NaN = NaN)
            for _ in range(4):
                wt0 = wtp.tile([128, 1024], bf16, tag="wt", bufs=4)
                nc.gpsimd.memset(wt0[:], 0.0)

            q_sb = keep.tile([128, QH, S], bf16)     # roped q per head (p=hd channel)
            k_sb = keep.tile([128, S], bf16)         # roped k
            vT_sb = keep.tile([128, JB, 128], bf16)  # v transposed blocks (p=key j)
            o_sb = keep.tile([128, QH, S], bf16)     # attention outputs
            wo_s = cw.tile([128, EB, OCB, 128], bf16)

            cc_in_h = [dramp.tile([128, S], bf16, name=f"ccin{hi}") for hi in range(QH)]
            cc_out_h = [dramp.tile([NCORES, 128, S], bf16, addr_space="Shared", name=f"ccout{hi}")
                        for hi in range(QH)]
            # dummy warm-up AllGather absorbs the ~11us first-collective init
            cc_warm_in = dramp.tile([128, 8], bf16, name="ccwi")
            cc_warm_out = dramp.tile([NCORES, 128, 8], bf16, addr_space="Shared", name="ccwo")

            def emit_attn_chunk(hi, t, scpool, appool, supool, filler, pop, sc_bufs=2):
                sl = slice(t * 512, (t + 1) * 512)
                njb = 4 * t + 4
                npairs = njb // 2
                attn_ps = appool.tile([128, 512], f32, tag="attn", bufs=1, name=f"at{hi}{t}")
                sums_ps = supool.tile([128, 512], f32, tag="sums", bufs=1, name=f"su{hi}{t}")

                def adv(n):
                    if filler is None:
                        return
                    for _ in range(n):
                        try:
                            next(filler)
                        except StopIteration:
                            break

                # diag blocks: only columns >= 128*d are unmasked; scores, exp
                # and AV all shrink to the live columns. Dead w_t columns stay
                # 0.0 (buffers zeroed once at startup; dmask keeps them 0).
                def c0(jb):
                    d = jb - 4 * t
                    return 0 if d < 0 else 128 * d

                def emit_scores(pj):
                    jb0 = 2 * pj
                    sc_ps = scpool.tile([128, 1024], f32, tag="sc", bufs=sc_bufs,
                                        name=f"sc{hi}{t}{pj}")
                    w_t = wtp.tile([128, 1024], bf16, tag="wt", bufs=4, name=f"wt{hi}{t}{pj}")
                    for u in range(2):
                        jb = jb0 + u
                        cu = u * 512 + c0(jb)
                        nc.tensor.matmul(sc_ps[:, cu:(u + 1) * 512],
                                         k_sb[:, jb * 128:(jb + 1) * 128],
                                         q_sb[:, hi, t * 512 + c0(jb):(t + 1) * 512],
                                         start=True, stop=True)
                        # exp per 512-half: halves the ACT latency AV waits on
                        nc.scalar.activation(w_t[:, cu:(u + 1) * 512],
                                             sc_ps[:, cu:(u + 1) * 512],
                                             mybir.ActivationFunctionType.Exp, scale=SCALE)
                    return w_t

                def emit_av(pj, w_t):
                    jb0 = 2 * pj
                    for u in range(2):
                        jb = jb0 + u
                        d = jb - 4 * t
                        cc = u * 512 + c0(jb)
                        if d >= 0:
                            # full-width mask so the fold below reads zeros in
                            # the masked region of this half
                            nc.vector.tensor_mul(w_t[:, u * 512:(u + 1) * 512],
                                                 w_t[:, u * 512:(u + 1) * 512],
                                                 dmask[:, d, 0:512])
                        st, sp = (jb == 0), (jb == njb - 1)
                        nc.tensor.matmul(attn_ps[:, c0(jb):512], vT_sb[:, jb, :], w_t[:, cc:(u + 1) * 512],
                                         start=st, stop=sp, skip_group_check=True)
                    # fold the pair's two key-blocks for the denominator
                    wf = ftmp.tile([128, 512], bf16, tag="wf", bufs=5, name=f"wf{hi}{t}{pj}")
                    nc.vector.tensor_add(wf[:], w_t[:, 0:512], w_t[:, 512:1024])
                    return wf

                # software-pipeline by one pair: the next pair's score MMs and
                # popped filler quanta sit between this pair's exp and its AV.
                # Denominator: DVE-fold 4 pairs (8 key-blocks) per ones-matmul.
                groups = [4] * (npairs // 4) + ([npairs % 4] if npairs % 4 else [])
                ngroups = len(groups)
                gi = 0
                folds = []

                def flush_folds(pj):
                    nonlocal gi, folds
                    if len(folds) != groups[gi]:
                        return
                    if len(folds) == 4:
                        ga = ftmp.tile([128, 512], bf16, tag="wg", bufs=2, name=f"ga{hi}{t}{pj}")
                        nc.vector.tensor_add(ga[:], folds[0][:], folds[1][:])
                        gb = ftmp.tile([128, 512], bf16, tag="wg", bufs=2, name=f"gb{hi}{t}{pj}")
                        nc.vector.tensor_add(gb[:], folds[2][:], folds[3][:])
                        gg = ftmp.tile([128, 512], bf16, tag="wg2", bufs=2, name=f"gg{hi}{t}{pj}")
                        nc.vector.tensor_add(gg[:], ga[:], gb[:])
                    else:
                        gg = ftmp.tile([128, 512], bf16, tag="wg2", bufs=2, name=f"gg{hi}{t}{pj}")
                        nc.vector.tensor_add(gg[:], folds[0][:], folds[1][:])
                    nc.tensor.matmul(sums_ps[:], ones[:], gg[:],
                                     start=(gi == 0), stop=(gi == ngroups - 1),
                                     skip_group_check=True)
                    gi += 1
                    folds = []

                w_prev = emit_scores(0)
                for pj in range(1, npairs):
                    adv(pop)
                    w_cur = emit_scores(pj)
                    folds.append(emit_av(pj - 1, w_prev))
                    w_prev = w_cur
                    flush_folds(pj)
                adv(pop)
                folds.append(emit_av(npairs - 1, w_prev))
                flush_folds(npairs)
                recip = ftmp.tile([128, 512], f32, tag="recip", name=f"re{hi}{t}")
                nc.vector.reciprocal_approx_fast(recip[:], sums_ps[:])
                nc.vector.tensor_mul(o_sb[:, hi, sl], attn_ps[:], recip[:])
                nc.gpsimd.dma_start(cc_in_h[hi][:, sl], o_sb[:, hi, sl])
                if t == ST - 1:
                    nc.gpsimd.collective_compute(
                        "AllGather",
                        mybir.AluOpType.bypass,
                        replica_groups=[list(range(NCORES))],
                        ins=[cc_in_h[hi][:]],
                        outs=[cc_out_h[hi][:]],
                    )

            with (
                tc.tile_pool(name="aw", bufs=1) as aw,
                tc.tile_pool(name="ax", bufs=2) as ax,
                tc.tile_pool(name="atmp", bufs=2) as atmp,
            ):
                wk_s = aw.tile([128, EB, 128], bf16)
                wv_s = aw.tile([128, EB, 128], bf16)
                wq_s = aw.tile([128, EB, QH, 128], bf16)

                def rope_copy(src):
                    # qb = bf16(psum) on ScalarE — frees the PSUM bank; emit
                    # all of a tile's copies back-to-back so banks free fast
                    qb = atmp.tile([128, 512], bf16, tag="qb", bufs=3)
                    nc.scalar.activation(qb[:], src[:], mybir.ActivationFunctionType.Copy)
                    return qb

                def rope_rest(qb, dst, sl):
                    # rot = swap_halves(qb) via SBUF DMA (gpsimd queue — the
                    # sync queue carries MB-sized loads that would delay it),
                    # then dst = qb*cos + rot*sinmod (sin sign-folded host-side)
                    rot = atmp.tile([128, 512], bf16, tag="rot")
                    nc.gpsimd.dma_start(rot[0:64, :], qb[64:128, :])
                    nc.gpsimd.dma_start(rot[64:128, :], qb[0:64, :])
                    qc = atmp.tile([128, 512], bf16, tag="qc")
                    nc.vector.tensor_mul(qc[:], qb[:], cos_s[:, sl])
                    rs = atmp.tile([128, 512], bf16, tag="rs")
                    nc.vector.tensor_mul(rs[:], rot[:], sin_s[:, sl])
                    nc.vector.tensor_add(dst[:, sl], qc[:], rs[:])

                def emit_rope(src, dst, sl):
                    rope_rest(rope_copy(src), dst, sl)

                # ---------------- Phase A1: k, v, q0, q1 projections ----------------
                with tc.tile_pool(name="a1psum", bufs=1, space="PSUM") as a1psum:
                    # t=0 loads split finely: x/wk/wv/wq01 interleaved ascending-eb
                    xch0 = []
                    for cix in range(4):
                        xc = ax.tile([128, 8, 512], bf16, tag="x", bufs=5, name=f"xc0_{cix}")
                        xch0.append(xc)
                    # cold start: x chunks on the sync queue, weights on the
                    # (idle) scalar and vector queues — three DMA queues drain
                    # the 7 MiB in parallel instead of serializing
                    nsub = [4, 2, 1, 1]  # pieces per 8-eb chunk (finer early)
                    for cix in range(4):
                        n = nsub[cix]
                        w = 8 // n
                        for u in range(n):
                            ss = slice(cix * 8 + u * w, cix * 8 + (u + 1) * w)
                            ls = slice(u * w, (u + 1) * w)
                            nc.sync.dma_start(xch0[cix][:, ls, :], xp[0, :, ss, :])
                            nc.scalar.dma_start(wk_s[:, ss, :], wkt[:, ss, :])
                            nc.scalar.dma_start(wv_s[:, ss, :], wvt[:, ss, :])
                    # q0/q1 weights behind wk/wv on the scalar queue (needed
                    # from the second t0 pass, ~17us in)
                    for cix in range(4):
                        ss = slice(cix * 8, (cix + 1) * 8)
                        nc.scalar.dma_start(wq_s[:, ss, 0:2, :], wqt[:, ss, 0:2, :])
                    nc.gpsimd.dma_start(cos_s[:], cosd[:])
                    nc.gpsimd.dma_start(sin_s[:], sind[:])
                    nc.gpsimd.dma_start(bv_s[:], bvp[:])
                    nc.gpsimd.dma_start(bo_s[:], bop[:])
                    nc.gpsimd.dma_start(cc_warm_in[:], ones[:, 0:8])
                    nc.gpsimd.collective_compute(
                        "AllGather",
                        mybir.AluOpType.bypass,
                        replica_groups=[list(range(NCORES))],
                        ins=[cc_warm_in[:]],
                        outs=[cc_warm_out[:]],
                    )

                    a2_pre = []
                    for t in range(ST):
                        sl = slice(t * 512, (t + 1) * 512)
                        if t == 0:
                            xch = xch0
                        else:
                            xch = []
                            for cix in range(4):
                                xc = ax.tile([128, 8, 512], bf16, tag="x", bufs=5)
                                nc.sync.dma_start(xc[:], xp[t, :, cix * 8:(cix + 1) * 8, :])
                                xch.append(xc)
                            if t == ST - 1:
                                # prefetch A2's t0 x + wq2/3 during A1-t3 so the
                                # a2 filler quanta inside attn(h0) never stall;
                                # interleaved per-octet in consumption order
                                for cix in range(4):
                                    ss = slice(cix * 8, (cix + 1) * 8)
                                    xc = ax.tile([128, 8, 512], bf16, tag="x", bufs=5,
                                                 name=f"a2x0_{cix}")
                                    a2_pre.append(xc)
                                for cix in range(4):
                                    ss = slice(cix * 8, (cix + 1) * 8)
                                    nc.sync.dma_start(a2_pre[cix][:], xp[0, :, ss, :])
                                    nc.sync.dma_start(wq_s[:, ss, 2:QH, :], wqt[:, ss, 2:QH, :])
                        k_ps = a1psum.tile([128, 512], f32, tag="k", bufs=2, name=f"k{t}")
                        v_ps = a1psum.tile([128, 512], f32, tag="v", bufs=2, name=f"v{t}")
                        q0_ps = a1psum.tile([128, 512], f32, tag="q0", name=f"q0{t}")
                        q1_ps = a1psum.tile([128, 512], f32, tag="q1", name=f"q1{t}")
                        if t == 0:
                            # two passes over the resident x: k/v first (half the
                            # cold-start weight bytes), then q0/q1
                            for b in range(EB):
                                st, sp = (b == 0), (b == EB - 1)
                                xb = xch[b // 8][:, b % 8, :]
                                nc.tensor.matmul(k_ps[:], wk_s[:, b, :], xb, start=st, stop=sp)
                                nc.tensor.matmul(v_ps[:], wv_s[:, b, :], xb, start=st, stop=sp)
                        else:
                            for b in range(EB):
                                st, sp = (b == 0), (b == EB - 1)
                                xb = xch[b // 8][:, b % 8, :]
                                nc.tensor.matmul(k_ps[:], wk_s[:, b, :], xb, start=st, stop=sp)
                                nc.tensor.matmul(v_ps[:], wv_s[:, b, :], xb, start=st, stop=sp)
                                nc.tensor.matmul(q0_ps[:], wq_s[:, b, 0, :], xb, start=st, stop=sp)
                                nc.tensor.matmul(q1_ps[:], wq_s[:, b, 1, :], xb, start=st, stop=sp)
                        # v: bias add (DVE), then PE transposes
                        v_sb = atmp.tile([128, 512], bf16, tag="v")
                        nc.vector.tensor_scalar_add(v_sb[:], v_ps[:], bv_s[:, 0:1])
                        tr_ps = a1psum.tile([128, 4, 128], bf16, tag="tr")
                        for i in range(4):
                            nc.tensor.transpose(tr_ps[:, i, :], v_sb[:, i * 128:(i + 1) * 128], ident[:])
                        nc.vector.tensor_copy(vT_sb[:, 4 * t:4 * t + 4, :], tr_ps[:])
                        if t == 0:
                            emit_rope(k_ps, k_sb, sl)
                            for b in range(EB):
                                st, sp = (b == 0), (b == EB - 1)
                                xb = xch[b // 8][:, b % 8, :]
                                nc.tensor.matmul(q0_ps[:], wq_s[:, b, 0, :], xb, start=st, stop=sp)
                                nc.tensor.matmul(q1_ps[:], wq_s[:, b, 1, :], xb, start=st, stop=sp)
                            emit_rope(q0_ps, q_sb[:, 0, :], sl)
                            emit_rope(q1_ps, q_sb[:, 1, :], sl)
                        else:
                            # front-load the ACT copies so all three PSUM
                            # accumulators free before the serial rot/mul chains
                            qbk = rope_copy(k_ps)
                            qb0 = rope_copy(q0_ps)
                            qb1 = rope_copy(q1_ps)
                            rope_rest(qbk, k_sb, sl)
                            rope_rest(qb0, q_sb[:, 0, :], sl)
                            rope_rest(qb1, q_sb[:, 1, :], sl)

                # ---- attn(h0/h1) with A2 (q2/q3) quanta as PE filler ----
                def a2_gen(a2psum):
                    for t in range(ST):
                        sl = slice(t * 512, (t + 1) * 512)
                        if t == 0:
                            xch = a2_pre  # prefetched during A1-t3
                        else:
                            xch = []
                            for cix in range(4):
                                xc = ax.tile([128, 8, 512], bf16, tag="x", bufs=5)
                                nc.sync.dma_start(xc[:], xp[t, :, cix * 8:(cix + 1) * 8, :])
                                xch.append(xc)
                        if t == ST - 1:
                            # wo load (4 MiB) behind the last x tile on the
                            # sync queue; needed ~100us later at o_proj
                            for cix in range(4):
                                nc.sync.dma_start(wo_s[:, cix * 8:(cix + 1) * 8, :, :],
                                                  wot[:, cix * 8:(cix + 1) * 8, :, :])
                        yield
                        ps = [a2psum.tile([128, 512], f32, tag=f"q{hi}", name=f"a2p{t}{hi}")
                              for hi in range(2, QH)]
                        for b in range(EB):
                            st, sp = (b == 0), (b == EB - 1)
                            xb = xch[b // 8][:, b % 8, :]
                            for i in range(2):
                                nc.tensor.matmul(ps[i][:], wq_s[:, b, 2 + i, :], xb,
                                                 start=st, stop=sp)
                            if b % 2 == 1:
                                yield
                        for i in range(2):
                            emit_rope(ps[i], q_sb[:, 2 + i, :], sl)
                            yield

                with (
                    tc.tile_pool(name="bpsum", bufs=1, space="PSUM") as bpsum,
                    tc.tile_pool(name="a2psum", bufs=1, space="PSUM") as a2psum,
                ):
                    a2 = a2_gen(a2psum)
                    next(a2)  # kick off wq23 + x t0 DMAs
                    for t in range(ST):
                        emit_attn_chunk(0, t, bpsum, bpsum, bpsum, a2, 2)
                    for t in range(ST):
                        emit_attn_chunk(1, t, bpsum, bpsum, bpsum, a2, 1)
                    for t in range(ST):
                        emit_attn_chunk(2, t, bpsum, bpsum, bpsum, a2, 2)
                    for _ in a2:  # drain any A2 leftovers before h3
                        pass

            # ------- attn(h3) + o_proj -------
            with (
                tc.tile_pool(name="b3psum", bufs=1, space="PSUM") as b3psum,
                tc.tile_pool(name="cg", bufs=2) as cg,
                tc.tile_pool(name="cout", bufs=1) as cout,
                tc.tile_pool(name="cpsum", space="PSUM", bufs=2) as cpsum,
            ):
                out_acc = cout.tile([128, ST, OCB, 512], f32)

                def oproj_gen(hi):
                    for t in range(ST):
                        ogc = cg.tile([128, NCORES, 512], bf16, tag="og", bufs=6, name=f"og{hi}{t}")
                        nc.sync.dma_start(
                            ogc[:], cc_out_h[hi][:, :, t * 512:(t + 1) * 512].transpose([1, 0, 2]))
                        yield
                        for o in range(OCB):
                            o_ps = cpsum.tile([128, 512], f32, tag="ops", name=f"op{hi}{t}{o}")
                            for r in range(NCORES):
                                nc.tensor.matmul(o_ps[:], wo_s[:, hi * NCORES + r, o, :], ogc[:, r, :],
                                                 start=(r == 0), stop=(r == NCORES - 1),
                                                 skip_group_check=True)
                                if r == 3:
                                    yield
                            if hi == 0:
                                nc.vector.tensor_copy(out_acc[:, t, o, :], o_ps[:])
                            else:
                                nc.vector.tensor_add(out_acc[:, t, o, :], out_acc[:, t, o, :], o_ps[:])
                            if hi == QH - 1:
                                nc.vector.tensor_scalar_add(out_acc[:, t, o, :], out_acc[:, t, o, :], bo_s[:, o:o + 1])
                                nc.sync.dma_start(out[o, t], out_acc[:, t, o, :])
                            yield

                op0 = oproj_gen(0)
                # h3 (t>=2 only, for AG0 rendezvous margin): fill with
                # oproj(h0) quanta; if skew is extreme the stall here matches
                # the stall the o_proj phase would eat anyway.
                for t in range(ST):
                    emit_attn_chunk(3, t, b3psum, b3psum, b3psum,
                                    op0 if t >= 2 else None, 1)
                for _ in op0:
                    pass
                for hi in range(1, QH):
                    for _ in oproj_gen(hi):
                        pass

    nc.finalize()
    return nc


def _pack_inputs(inputs):
    """Host-side shard + pack into DMA-friendly per-core layouts."""
    x = np.asarray(inputs["input_embeds"], np.float32).reshape(E, S)
    cos = np.asarray(inputs["cos"], np.float32)
    sin = np.asarray(inputs["sin"], np.float32)
    wq = np.asarray(inputs["wq"], np.float32)
    wk = np.asarray(inputs["wk"], np.float32)
    wv = np.asarray(inputs["wv"], np.float32)
    bv = np.asarray(inputs["bv"], np.float32)
    wo = np.asarray(inputs["wo"], np.float32)
    bo = np.asarray(inputs["bo"], np.float32)

    sinmod = np.concatenate([-sin[:64], sin[64:]], axis=0)

    # x packed: [ST, 128, EB, 512]; xp[t, p, b, s] = x[b*128+p, t*512+s]
    xp = np.ascontiguousarray(
        x.reshape(EB, 128, ST, 512).transpose(2, 1, 0, 3)
    ).astype(_BF16)

    # gathered-channel permutation for wo columns (hi-major after per-head AG):
    # g = hi*1024 + r*128 + d -> original channel (r + 8*hi)*128 + d
    g = np.arange(E)
    hi, rem = g // (NCORES * 128), g % (NCORES * 128)
    r, d = rem // 128, rem % 128
    colperm = (r + NCORES * hi) * 128 + d

    in_maps = []
    for c in range(NCORES):
        qheads = [c + NCORES * i for i in range(QH)]
        # wqt[p, b, hi, m] = wq[head*128+m, b*128+p]
        wq_loc = wq[np.concatenate([np.arange(h * 128, (h + 1) * 128) for h in qheads])]  # [512, E]
        wqt = np.ascontiguousarray(
            wq_loc.reshape(QH, 128, EB, 128).transpose(3, 2, 0, 1)
        ).astype(_BF16)
        wk_loc = wk[c * 128:(c + 1) * 128]  # [128, E]
        wkt = np.ascontiguousarray(
            wk_loc.reshape(128, EB, 128).transpose(2, 1, 0)
        ).astype(_BF16)
        wv_loc = wv[c * 128:(c + 1) * 128]
        wvt = np.ascontiguousarray(
            wv_loc.reshape(128, EB, 128).transpose(2, 1, 0)
        ).astype(_BF16)
        # wot[p, b, o, m] = wo[c*512 + o*128 + m, colperm[b*128+p]]
        wo_loc = wo[c * 512:(c + 1) * 512][:, colperm]  # [512, E] permuted cols
        wot = np.ascontiguousarray(
            wo_loc.reshape(OCB, 128, EB, 128).transpose(3, 2, 0, 1)
        ).astype(_BF16)
        in_maps.append({
            "xp": xp,
            "wqt": wqt, "wkt": wkt, "wvt": wvt, "wot": wot,
            "cosd": cos.astype(_BF16), "sind": sinmod.astype(_BF16),
            "bvp": np.ascontiguousarray(bv[c * 128:(c + 1) * 128].reshape(128, 1)),
            "bop": np.ascontiguousarray(bo[c * 512:(c + 1) * 512].reshape(OCB, 128).T),
        })
    return in_maps


def _install_ntff_hook():
    """The agent image lacks antenv.axon_hooks; recreate it so trace=True
    (neuron-profile exec_time_ns) works under axon."""
    import sys, types
    try:
        from antenv.axon_hooks import get_axon_ntff_profile_hook  # noqa
        return
    except ImportError:
        pass
    mod = types.ModuleType("antenv.axon_hooks")
    _h = [None]
    mod.set_axon_ntff_profile_hook = lambda h: _h.__setitem__(0, h)
    mod.get_axon_ntff_profile_hook = lambda: _h[0]
    sys.modules["antenv.axon_hooks"] = mod
    import antenv
    antenv.axon_hooks = mod
    try:
        from trn_agent_boot.trn_boot import _ntff_profile_via_ctypes
        mod.set_axon_ntff_profile_hook(
            _ntff_profile_via_ctypes("/opt/axon/libaxon_pjrt.so"))
    except Exception:
        pass


def kernel(_profile=False, **inputs):
    global _COMPILED, LAST_EXEC_NS
    from concourse.bass_utils import run_bass_kernel_spmd

    if _profile:
        _install_ntff_hook()

    if _COMPILED is None:
        _COMPILED = _build_graph()
    nc = _COMPILED

    in_maps = _pack_inputs(inputs)
    res = run_bass_kernel_spmd(nc, in_maps, core_ids=list(range(NCORES)), trace=_profile)
    if _profile:
        LAST_EXEC_NS = res.exec_time_ns
        global LAST_INSTS
        LAST_INSTS = res.instructions_and_trace
    outs = res.results

    full = np.empty((E, S), np.float32)
    for c in range(NCORES):
        oc = np.asarray(outs[c]["out"], np.float32)  # [OCB, ST, 128, 512]
        full[c * 512:(c + 1) * 512] = oc.transpose(0, 2, 1, 3).reshape(512, S)
    return full.reshape(B, E, 1, S)


# revision 25
# speedup vs baseline: 1.0194x; 1.0194x over previous
"""GQA attention (B=1, E=4096, H=32, KVH=8, S=2048, HD=128) on 8 TRN2 cores.

Sharding: tensor-parallel over heads. Core c owns q heads {c, c+8, c+16, c+24}
(all attend to kv head c under the reference's channel-block-repeat GQA
tiling): 4 q-head projections + 1 kv-head k/v projection + RoPE + causal
attention are core-local. Attention outputs are AllGathered per head
(2 MiB/rank bf16); each core then computes a 512-row slice of o_proj.

Schedule (v3): keep the PE stream dense and post collectives early so the
per-rank SPMD launch skew (observed 50-100us) is absorbed by queued
independent work instead of PE idle:
  A1: k/v/q0/q1 projections (4 PSUM accumulators) for all 4 seq tiles.
  attn(h0) then attn(h1), each interleaved with A2 (q2/q3 projection) quanta
    as PE filler between score/AV pairs -> AG(h0)/AG(h1) post ~80us earlier
    than an h-major schedule would.
  attn(h2) drains A2 leftovers; attn(h3) pops o_proj(h0) quanta (AG0 landed
    unless skew is extreme; then it stalls no worse than the tail would).
  o_proj drain h0..h3 with per-(t,o) out DMAs.
Exp runs as two [128,512] halves per pair (halves the ACT latency the AV
waits on). Softmax denominator: DVE pair-folds + one ones-matmul per 4
key-blocks. Scores+exp always run full-width so PSUM/w_t hold finite values
everywhere (0*NaN from masking stale memory poisoned the denominator once);
only the AV matmul shrinks masked columns.

Numerics: matmuls bf16 (f32 PSUM); softmax without max-subtraction (scores are
O(5)); exp on ScalarE from PSUM with 1/sqrt(128) folded into activation scale.
"""

import numpy as np
import ml_dtypes

B, E, H, KVH, S = 1, 4096, 32, 8, 2048
HD = E // H            # 128
NCORES = 8
QH = H // NCORES       # 4 q heads per core
EB = E // 128          # 32 e-blocks
ST = S // 512          # 4 s-tiles of 512
JB = S // 128          # 16 j-blocks of 128
OCB = (E // NCORES) // 128  # 4 output-channel blocks per core (512 rows)

_BF16 = ml_dtypes.bfloat16

_COMPILED = None       # (nc, names) cache
LAST_EXEC_NS = None    # set when _profile=True
LAST_INSTS = None


def _build_graph():
    import concourse.bass as bass
    import concourse.bacc as bacc
    import concourse.mybir as mybir
    from concourse import tile
    from concourse.masks import make_identity

    f32 = mybir.dt.float32
    bf16 = mybir.dt.bfloat16
    SCALE = 1.0 / float(np.sqrt(HD))

    nc = bacc.Bacc("TRN2", target_bir_lowering=False, num_devices=NCORES)

    xp = nc.declare_dram_parameter("xp", [ST, 128, EB, 512], bf16, isOutput=False)
    wqt = nc.declare_dram_parameter("wqt", [128, EB, QH, 128], bf16, isOutput=False)
    wkt = nc.declare_dram_parameter("wkt", [128, EB, 128], bf16, isOutput=False)
    wvt = nc.declare_dram_parameter("wvt", [128, EB, 128], bf16, isOutput=False)
    wot = nc.declare_dram_parameter("wot", [128, EB, OCB, 128], bf16, isOutput=False)
    cosd = nc.declare_dram_parameter("cosd", [128, S], bf16, isOutput=False)
    sind = nc.declare_dram_parameter("sind", [128, S], bf16, isOutput=False)
    bvp = nc.declare_dram_parameter("bvp", [128, 1], f32, isOutput=False)
    bop = nc.declare_dram_parameter("bop", [128, OCB], f32, isOutput=False)
    out = nc.declare_dram_parameter("out", [OCB, ST, 128, 512], f32, isOutput=True)

    with tile.TileContext(nc) as tc:
        with (
            tc.tile_pool(name="const", bufs=1) as constp,
            tc.tile_pool(name="keep", bufs=1) as keep,
            tc.tile_pool(name="wtp", bufs=1) as wtp,
            tc.tile_pool(name="ftmp", bufs=2) as ftmp,
            tc.tile_pool(name="cw", bufs=1) as cw,
            tc.tile_pool(name="dramp", bufs=1, space="DRAM") as dramp,
        ):
            ident = constp.tile([128, 128], bf16)
            make_identity(nc, ident)
            ones = constp.tile([128, 128], bf16)
            nc.gpsimd.memset(ones[:], 1.0)
            # diag masks: mask_d[j, s] = 1 if s >= 128*d + j else 0
            dmask = constp.tile([128, 4, 512], bf16)
            for d in range(4):
                nc.gpsimd.memset(dmask[:, d, :], 1.0)
                nc.gpsimd.affine_select(
                    out=dmask[:, d, :], in_=dmask[:, d, :],
                    compare_op=mybir.AluOpType.is_ge,
                    fill=0.0, base=-128 * d,
                    pattern=[[1, 512]], channel_multiplier=-1,
                )
            cos_s = constp.tile([128, S], bf16)
            sin_s = constp.tile([128, S], bf16)
            bv_s = constp.tile([128, 1], f32)
            bo_s = constp.tile([128, OCB], f32)
            # zero the 4 rotating w_t buffers once: masked/never-exp'd columns
            # must read as 0.0 (stale SBUF can hold NaN bit patterns and the
            # dmask multiply would turn them into 0*NaN = NaN)
            for _ in range(4):
                wt0 = wtp.tile([128, 1024], bf16, tag="wt", bufs=4)
                nc.gpsimd.memset(wt0[:], 0.0)

            q_sb = keep.tile([128, QH, S], bf16)     # roped q per head (p=hd channel)
            k_sb = keep.tile([128, S], bf16)         # roped k
            vT_sb = keep.tile([128, JB, 128], bf16)  # v transposed blocks (p=key j)
            o_sb = keep.tile([128, QH, S], bf16)     # attention outputs
            wo_s = cw.tile([128, EB, OCB, 128], bf16)

            cc_in_h = [dramp.tile([128, S], bf16, name=f"ccin{hi}") for hi in range(QH)]
            cc_out_h = [dramp.tile([NCORES, 128, S], bf16, addr_space="Shared", name=f"ccout{hi}")
                        for hi in range(QH)]
            # dummy warm-up AllGather absorbs the ~11us first-collective init
            cc_warm_in = dramp.tile([128, 8], bf16, name="ccwi")
            cc_warm_out = dramp.tile([NCORES, 128, 8], bf16, addr_space="Shared", name="ccwo")

            def emit_attn_chunk(hi, t, scpool, appool, supool, filler, pop, sc_bufs=2):
                sl = slice(t * 512, (t + 1) * 512)
                njb = 4 * t + 4
                npairs = njb // 2
                attn_ps = appool.tile([128, 512], f32, tag="attn", bufs=1, name=f"at{hi}{t}")
                sums_ps = supool.tile([128, 512], f32, tag="sums", bufs=1, name=f"su{hi}{t}")

                def adv(n):
                    if filler is None:
                        return
                    for _ in range(n):
                        try:
                            next(filler)
                        except StopIteration:
                            break

                # diag blocks: only columns >= 128*d are unmasked; scores, exp
                # and AV all shrink to the live columns. Dead w_t columns stay
                # 0.0 (buffers zeroed once at startup; dmask keeps them 0).
                def c0(jb):
                    d = jb - 4 * t
                    return 0 if d < 0 else 128 * d

                def emit_scores(pj):
                    jb0 = 2 * pj
                    sc_ps = scpool.tile([128, 1024], f32, tag="sc", bufs=sc_bufs,
                                        name=f"sc{hi}{t}{pj}")
                    w_t = wtp.tile([128, 1024], bf16, tag="wt", bufs=4, name=f"wt{hi}{t}{pj}")
                    for u in range(2):
                        jb = jb0 + u
                        cu = u * 512 + c0(jb)
                        nc.tensor.matmul(sc_ps[:, cu:(u + 1) * 512],
                                         k_sb[:, jb * 128:(jb + 1) * 128],
                                         q_sb[:, hi, t * 512 + c0(jb):(t + 1) * 512],
                                         start=True, stop=True)
                        # exp per 512-half: halves the ACT latency AV waits on
                        nc.scalar.activation(w_t[:, cu:(u + 1) * 512],
                                             sc_ps[:, cu:(u + 1) * 512],
                                             mybir.ActivationFunctionType.Exp, scale=SCALE)
                    return w_t

                def emit_av(pj, w_t):
                    jb0 = 2 * pj
                    for u in range(2):
                        jb = jb0 + u
                        d = jb - 4 * t
                        cc = u * 512 + c0(jb)
                        if d >= 0:
                            # full-width mask so the fold below reads zeros in
                            # the masked region of this half
                            nc.vector.tensor_mul(w_t[:, u * 512:(u + 1) * 512],
                                                 w_t[:, u * 512:(u + 1) * 512],
                                                 dmask[:, d, 0:512])
                        st, sp = (jb == 0), (jb == njb - 1)
                        nc.tensor.matmul(attn_ps[:, c0(jb):512], vT_sb[:, jb, :], w_t[:, cc:(u + 1) * 512],
                                         start=st, stop=sp, skip_group_check=True)
                    # fold the pair's two key-blocks for the denominator
                    wf = ftmp.tile([128, 512], bf16, tag="wf", bufs=5, name=f"wf{hi}{t}{pj}")
                    nc.vector.tensor_add(wf[:], w_t[:, 0:512], w_t[:, 512:1024])
                    return wf

                # software-pipeline by one pair: the next pair's score MMs and
                # popped filler quanta sit between this pair's exp and its AV.
                # Denominator: DVE-fold 4 pairs (8 key-blocks) per ones-matmul.
                groups = [4] * (npairs // 4) + ([npairs % 4] if npairs % 4 else [])
                ngroups = len(groups)
                gi = 0
                folds = []

                def flush_folds(pj):
                    nonlocal gi, folds
                    if len(folds) != groups[gi]:
                        return
                    if len(folds) == 4:
                        ga = ftmp.tile([128, 512], bf16, tag="wg", bufs=2, name=f"ga{hi}{t}{pj}")
                        nc.vector.tensor_add(ga[:], folds[0][:], folds[1][:])
                        gb = ftmp.tile([128, 512], bf16, tag="wg", bufs=2, name=f"gb{hi}{t}{pj}")
                        nc.vector.tensor_add(gb[:], folds[2][:], folds[3][:])
                        gg = ftmp.tile([128, 512], bf16, tag="wg2", bufs=2, name=f"gg{hi}{t}{pj}")
                        nc.vector.tensor_add(gg[:], ga[:], gb[:])
                    else:
                        gg = ftmp.tile([128, 512], bf16, tag="wg2", bufs=2, name=f"gg{hi}{t}{pj}")
                        nc.vector.tensor_add(gg[:], folds[0][:], folds[1][:])
                    nc.tensor.matmul(sums_ps[:], ones[:], gg[:],
                                     start=(gi == 0), stop=(gi == ngroups - 1),
                                     skip_group_check=True)
                    gi += 1
                    folds = []

                w_prev = emit_scores(0)
                for pj in range(1, npairs):
                    adv(pop)
                    w_cur = emit_scores(pj)
                    folds.append(emit_av(pj - 1, w_prev))
                    w_prev = w_cur
                    flush_folds(pj)
                adv(pop)
                folds.append(emit_av(npairs - 1, w_prev))
                flush_folds(npairs)
                recip = ftmp.tile([128, 512], f32, tag="recip", name=f"re{hi}{t}")
                nc.vector.reciprocal_approx_fast(recip[:], sums_ps[:])
                nc.vector.tensor_mul(o_sb[:, hi, sl], attn_ps[:], recip[:])
                nc.gpsimd.dma_start(cc_in_h[hi][:, sl], o_sb[:, hi, sl])
                if t == ST - 1:
                    nc.gpsimd.collective_compute(
                        "AllGather",
                        mybir.AluOpType.bypass,
                        replica_groups=[list(range(NCORES))],
                        ins=[cc_in_h[hi][:]],
                        outs=[cc_out_h[hi][:]],
                    )

            with (
                tc.tile_pool(name="aw", bufs=1) as aw,
                tc.tile_pool(name="ax", bufs=2) as ax,
                tc.tile_pool(name="atmp", bufs=2) as atmp,
            ):
                wk_s = aw.tile([128, EB, 128], bf16)
                wv_s = aw.tile([128, EB, 128], bf16)
                wq_s = aw.tile([128, EB, QH, 128], bf16)

                def rope_copy(src):
                    # qb = bf16(psum) on ScalarE — frees the PSUM bank; emit
                    # all of a tile's copies back-to-back so banks free fast
                    qb = atmp.tile([128, 512], bf16, tag="qb", bufs=3)
                    nc.scalar.activation(qb[:], src[:], mybir.ActivationFunctionType.Copy)
                    return qb

                def rope_rest(qb, dst, sl):
                    # rot = swap_halves(qb) via SBUF DMA (gpsimd queue — the
                    # sync queue carries MB-sized loads that would delay it),
                    # then dst = qb*cos + rot*sinmod (sin sign-folded host-side)
                    rot = atmp.tile([128, 512], bf16, tag="rot")
                    nc.gpsimd.dma_start(rot[0:64, :], qb[64:128, :])
                    nc.gpsimd.dma_start(rot[64:128, :], qb[0:64, :])
                    qc = atmp.tile([128, 512], bf16, tag="qc")
                    nc.vector.tensor_mul(qc[:], qb[:], cos_s[:, sl])
                    rs = atmp.tile([128, 512], bf16, tag="rs")
                    nc.vector.tensor_mul(rs[:], rot[:], sin_s[:, sl])
                    nc.vector.tensor_add(dst[:, sl], qc[:], rs[:])

                def emit_rope(src, dst, sl):
                    rope_rest(rope_copy(src), dst, sl)

                # ---------------- Phase A1: k, v, q0, q1 projections ----------------
                with tc.tile_pool(name="a1psum", bufs=1, space="PSUM") as a1psum:
                    # t=0 loads split finely: x/wk/wv/wq01 interleaved ascending-eb
                    xch0 = []
                    for cix in range(4):
                        xc = ax.tile([128, 8, 512], bf16, tag="x", bufs=5, name=f"xc0_{cix}")
                        xch0.append(xc)
                    nsub = [4, 2, 1, 1]  # pieces per 8-eb chunk (finer early)
                    for cix in range(4):
                        n = nsub[cix]
                        w = 8 // n
                        for u in range(n):
                            ss = slice(cix * 8 + u * w, cix * 8 + (u + 1) * w)
                            ls = slice(u * w, (u + 1) * w)
                            nc.sync.dma_start(xch0[cix][:, ls, :], xp[0, :, ss, :])
                            nc.sync.dma_start(wk_s[:, ss, :], wkt[:, ss, :])
                            nc.sync.dma_start(wv_s[:, ss, :], wvt[:, ss, :])
                    # q0/q1 weights behind the k/v stream (needed from the
                    # second t0 pass, ~17us in)
                    for cix in range(4):
                        ss = slice(cix * 8, (cix + 1) * 8)
                        nc.sync.dma_start(wq_s[:, ss, 0:2, :], wqt[:, ss, 0:2, :])
                    nc.gpsimd.dma_start(cos_s[:], cosd[:])
                    nc.gpsimd.dma_start(sin_s[:], sind[:])
                    nc.gpsimd.dma_start(bv_s[:], bvp[:])
                    nc.gpsimd.dma_start(bo_s[:], bop[:])
                    nc.gpsimd.dma_start(cc_warm_in[:], ones[:, 0:8])
                    nc.gpsimd.collective_compute(
                        "AllGather",
                        mybir.AluOpType.bypass,
                        replica_groups=[list(range(NCORES))],
                        ins=[cc_warm_in[:]],
                        outs=[cc_warm_out[:]],
                    )

                    a2_pre = []
                    for t in range(ST):
                        sl = slice(t * 512, (t + 1) * 512)
                        if t == 0:
                            xch = xch0
                        else:
                            xch = []
                            for cix in range(4):
                                xc = ax.tile([128, 8, 512], bf16, tag="x", bufs=5)
                                nc.sync.dma_start(xc[:], xp[t, :, cix * 8:(cix + 1) * 8, :])
                                xch.append(xc)
                            if t == ST - 1:
                                # prefetch A2's t0 x + wq2/3 during A1-t3 so the
                                # a2 filler quanta inside attn(h0) never stall;
                                # interleaved per-octet in consumption order
                                for cix in range(4):
                                    ss = slice(cix * 8, (cix + 1) * 8)
                                    xc = ax.tile([128, 8, 512], bf16, tag="x", bufs=5,
                                                 name=f"a2x0_{cix}")
                                    a2_pre.append(xc)
                                for cix in range(4):
                                    ss = slice(cix * 8, (cix + 1) * 8)
                                    nc.sync.dma_start(a2_pre[cix][:], xp[0, :, ss, :])
                                    nc.sync.dma_start(wq_s[:, ss, 2:QH, :], wqt[:, ss, 2:QH, :])
                        k_ps = a1psum.tile([128, 512], f32, tag="k", bufs=2, name=f"k{t}")
                        v_ps = a1psum.tile([128, 512], f32, tag="v", bufs=2, name=f"v{t}")
                        q0_ps = a1psum.tile([128, 512], f32, tag="q0", name=f"q0{t}")
                        q1_ps = a1psum.tile([128, 512], f32, tag="q1", name=f"q1{t}")
                        if t == 0:
                            # two passes over the resident x: k/v first (half the
                            # cold-start weight bytes), then q0/q1
                            for b in range(EB):
                                st, sp = (b == 0), (b == EB - 1)
                                xb = xch[b // 8][:, b % 8, :]
                                nc.tensor.matmul(k_ps[:], wk_s[:, b, :], xb, start=st, stop=sp)
                                nc.tensor.matmul(v_ps[:], wv_s[:, b, :], xb, start=st, stop=sp)
                        else:
                            for b in range(EB):
                                st, sp = (b == 0), (b == EB - 1)
                                xb = xch[b // 8][:, b % 8, :]
                                nc.tensor.matmul(k_ps[:], wk_s[:, b, :], xb, start=st, stop=sp)
                                nc.tensor.matmul(v_ps[:], wv_s[:, b, :], xb, start=st, stop=sp)
                                nc.tensor.matmul(q0_ps[:], wq_s[:, b, 0, :], xb, start=st, stop=sp)
                                nc.tensor.matmul(q1_ps[:], wq_s[:, b, 1, :], xb, start=st, stop=sp)
                        # v: bias add (DVE), then PE transposes
                        v_sb = atmp.tile([128, 512], bf16, tag="v")
                        nc.vector.tensor_scalar_add(v_sb[:], v_ps[:], bv_s[:, 0:1])
                        tr_ps = a1psum.tile([128, 4, 128], bf16, tag="tr")
                        for i in range(4):
                            nc.tensor.transpose(tr_ps[:, i, :], v_sb[:, i * 128:(i + 1) * 128], ident[:])
                        nc.vector.tensor_copy(vT_sb[:, 4 * t:4 * t + 4, :], tr_ps[:])
                        if t == 0:
                            emit_rope(k_ps, k_sb, sl)
                            for b in range(EB):
                                st, sp = (b == 0), (b == EB - 1)
                                xb = xch[b // 8][:, b % 8, :]
                                nc.tensor.matmul(q0_ps[:], wq_s[:, b, 0, :], xb, start=st, stop=sp)
                                nc.tensor.matmul(q1_ps[:], wq_s[:, b, 1, :], xb, start=st, stop=sp)
                            emit_rope(q0_ps, q_sb[:, 0, :], sl)
                            emit_rope(q1_ps, q_sb[:, 1, :], sl)
                        else:
                            # front-load the ACT copies so all three PSUM
                            # accumulators free before the serial rot/mul chains
                            qbk = rope_copy(k_ps)
                            qb0 = rope_copy(q0_ps)
                            qb1 = rope_copy(q1_ps)
                            rope_rest(qbk, k_sb, sl)
                            rope_rest(qb0, q_sb[:, 0, :], sl)
                            rope_rest(qb1, q_sb[:, 1, :], sl)

                # ---- attn(h0/h1) with A2 (q2/q3) quanta as PE filler ----
                def a2_gen(a2psum):
                    for t in range(ST):
                        sl = slice(t * 512, (t + 1) * 512)
                        if t == 0:
                            xch = a2_pre  # prefetched during A1-t3
                        else:
                            xch = []
                            for cix in range(4):
                                xc = ax.tile([128, 8, 512], bf16, tag="x", bufs=5)
                                nc.sync.dma_start(xc[:], xp[t, :, cix * 8:(cix + 1) * 8, :])
                                xch.append(xc)
                        if t == ST - 1:
                            # wo load (4 MiB) behind the last x tile on the
                            # sync queue; needed ~100us later at o_proj
                            for cix in range(4):
                                nc.sync.dma_start(wo_s[:, cix * 8:(cix + 1) * 8, :, :],
                                                  wot[:, cix * 8:(cix + 1) * 8, :, :])
                        yield
                        ps = [a2psum.tile([128, 512], f32, tag=f"q{hi}", name=f"a2p{t}{hi}")
                              for hi in range(2, QH)]
                        for b in range(EB):
                            st, sp = (b == 0), (b == EB - 1)
                            xb = xch[b // 8][:, b % 8, :]
                            for i in range(2):
                                nc.tensor.matmul(ps[i][:], wq_s[:, b, 2 + i, :], xb,
                                                 start=st, stop=sp)
                            if b % 2 == 1:
                                yield
                        for i in range(2):
                            emit_rope(ps[i], q_sb[:, 2 + i, :], sl)
                            yield

                with (
                    tc.tile_pool(name="bpsum", bufs=1, space="PSUM") as bpsum,
                    tc.tile_pool(name="a2psum", bufs=1, space="PSUM") as a2psum,
                ):
                    a2 = a2_gen(a2psum)
                    next(a2)  # kick off wq23 + x t0 DMAs
                    for t in range(ST):
                        emit_attn_chunk(0, t, bpsum, bpsum, bpsum, a2, 2)
                    for t in range(ST):
                        emit_attn_chunk(1, t, bpsum, bpsum, bpsum, a2, 1)
                    for t in range(ST):
                        emit_attn_chunk(2, t, bpsum, bpsum, bpsum, a2, 2)
                    for _ in a2:  # drain any A2 leftovers before h3
                        pass

            # ------- attn(h3) + o_proj -------
            with (
                tc.tile_pool(name="b3psum", bufs=1, space="PSUM") as b3psum,
                tc.tile_pool(name="cg", bufs=2) as cg,
                tc.tile_pool(name="cout", bufs=1) as cout,
                tc.tile_pool(name="cpsum", space="PSUM", bufs=2) as cpsum,
            ):
                out_acc = cout.tile([128, ST, OCB, 512], f32)

                def oproj_gen(hi):
                    for t in range(ST):
                        ogc = cg.tile([128, NCORES, 512], bf16, tag="og", bufs=6, name=f"og{hi}{t}")
                        nc.sync.dma_start(
                            ogc[:], cc_out_h[hi][:, :, t * 512:(t + 1) * 512].transpose([1, 0, 2]))
                        yield
                        for o in range(OCB):
                            o_ps = cpsum.tile([128, 512], f32, tag="ops", name=f"op{hi}{t}{o}")
                            for r in range(NCORES):
                                nc.tensor.matmul(o_ps[:], wo_s[:, hi * NCORES + r, o, :], ogc[:, r, :],
                                                 start=(r == 0), stop=(r == NCORES - 1),
                                                 skip_group_check=True)
                                if r == 3:
                                    yield
                            if hi == 0:
                                nc.vector.tensor_copy(out_acc[:, t, o, :], o_ps[:])
                            else:
                                nc.vector.tensor_add(out_acc[:, t, o, :], out_acc[:, t, o, :], o_ps[:])
                            if hi == QH - 1:
                                nc.vector.tensor_scalar_add(out_acc[:, t, o, :], out_acc[:, t, o, :], bo_s[:, o:o + 1])
                                nc.sync.dma_start(out[o, t], out_acc[:, t, o, :])
                            yield

                op0 = oproj_gen(0)
                # h3 (t>=2 only, for AG0 rendezvous margin): fill with
                # oproj(h0) quanta; if skew is extreme the stall here matches
                # the stall the o_proj phase would eat anyway.
                for t in range(ST):
                    emit_attn_chunk(3, t, b3psum, b3psum, b3psum,
                                    op0 if t >= 2 else None, 1)
                for _ in op0:
                    pass
                for hi in range(1, QH):
                    for _ in oproj_gen(hi):
                        pass

    nc.finalize()
    return nc


def _pack_inputs(inputs):
    """Host-side shard + pack into DMA-friendly per-core layouts."""
    x = np.asarray(inputs["input_embeds"], np.float32).reshape(E, S)
    cos = np.asarray(inputs["cos"], np.float32)
    sin = np.asarray(inputs["sin"], np.float32)
    wq = np.asarray(inputs["wq"], np.float32)
    wk = np.asarray(inputs["wk"], np.float32)
    wv = np.asarray(inputs["wv"], np.float32)
    bv = np.asarray(inputs["bv"], np.float32)
    wo = np.asarray(inputs["wo"], np.float32)
    bo = np.asarray(inputs["bo"], np.float32)

    sinmod = np.concatenate([-sin[:64], sin[64:]], axis=0)

    # x packed: [ST, 128, EB, 512]; xp[t, p, b, s] = x[b*128+p, t*512+s]
    xp = np.ascontiguousarray(
        x.reshape(EB, 128, ST, 512).transpose(2, 1, 0, 3)
    ).astype(_BF16)

    # gathered-channel permutation for wo columns (hi-major after per-head AG):
    # g = hi*1024 + r*128 + d -> original channel (r + 8*hi)*128 + d
    g = np.arange(E)
    hi, rem = g // (NCORES * 128), g % (NCORES * 128)
    r, d = rem // 128, rem % 128
    colperm = (r + NCORES * hi) * 128 + d

    in_maps = []
    for c in range(NCORES):
        qheads = [c + NCORES * i for i in range(QH)]
        # wqt[p, b, hi, m] = wq[head*128+m, b*128+p]
        wq_loc = wq[np.concatenate([np.arange(h * 128, (h + 1) * 128) for h in qheads])]  # [512, E]
        wqt = np.ascontiguousarray(
            wq_loc.reshape(QH, 128, EB, 128).transpose(3, 2, 0, 1)
        ).astype(_BF16)
        wk_loc = wk[c * 128:(c + 1) * 128]  # [128, E]
        wkt = np.ascontiguousarray(
            wk_loc.reshape(128, EB, 128).transpose(2, 1, 0)
        ).astype(_BF16)
        wv_loc = wv[c * 128:(c + 1) * 128]
        wvt = np.ascontiguousarray(
            wv_loc.reshape(128, EB, 128).transpose(2, 1, 0)
        ).astype(_BF16)
        # wot[p, b, o, m] = wo[c*512 + o*128 + m, colperm[b*128+p]]
        wo_loc = wo[c * 512:(c + 1) * 512][:, colperm]  # [512, E] permuted cols
        wot = np.ascontiguousarray(
            wo_loc.reshape(OCB, 128, EB, 128).transpose(3, 2, 0, 1)
        ).astype(_BF16)
        in_maps.append({
            "xp": xp,
            "wqt": wqt, "wkt": wkt, "wvt": wvt, "wot": wot,
            "cosd": cos.astype(_BF16), "sind": sinmod.astype(_BF16),
            "bvp": np.ascontiguousarray(bv[c * 128:(c + 1) * 128].reshape(128, 1)),
            "bop": np.ascontiguousarray(bo[c * 512:(c + 1) * 512].reshape(OCB, 128).T),
        })
    return in_maps


def _install_ntff_hook():
    """The agent image lacks antenv.axon_hooks; recreate it so trace=True
    (neuron-profile exec_time_ns) works under axon."""
    import sys, types
    try:
        from antenv.axon_hooks import get_axon_ntff_profile_hook  # noqa
        return
    except ImportError:
        pass
    mod = types.ModuleType("antenv.axon_hooks")
    _h = [None]
    mod.set_axon_ntff_profile_hook = lambda h: _h.__setitem__(0, h)
    mod.get_axon_ntff_profile_hook = lambda: _h[0]
    sys.modules["antenv.axon_hooks"] = mod
    import antenv
    antenv.axon_hooks = mod
    try:
        from trn_agent_boot.trn_boot import _ntff_profile_via_ctypes
        mod.set_axon_ntff_profile_hook(
            _ntff_profile_via_ctypes("/opt/axon/libaxon_pjrt.so"))
    except Exception:
        pass


def kernel(_profile=False, **inputs):
    global _COMPILED, LAST_EXEC_NS
    from concourse.bass_utils import run_bass_kernel_spmd

    if _profile:
        _install_ntff_hook()

    if _COMPILED is None:
        _COMPILED = _build_graph()
    nc = _COMPILED

    in_maps = _pack_inputs(inputs)
    res = run_bass_kernel_spmd(nc, in_maps, core_ids=list(range(NCORES)), trace=_profile)
    if _profile:
        LAST_EXEC_NS = res.exec_time_ns
        global LAST_INSTS
        LAST_INSTS = res.instructions_and_trace
    outs = res.results

    full = np.empty((E, S), np.float32)
    for c in range(NCORES):
        oc = np.asarray(outs[c]["out"], np.float32)  # [OCB, ST, 128, 512]
        full[c * 512:(c + 1) * 512] = oc.transpose(0, 2, 1, 3).reshape(512, S)
    return full.reshape(B, E, 1, S)
